# revision 12
# baseline (speedup 1.0000x reference)
"""Trainium2 Bass kernel for nn_AttentionBlock (B=4, C=512, T=2048, H=8, G=32).

Sharding: 8 cores = 4 batches x 2 head-groups (4 heads each).
Per core (batch b, head-group hg):
  h = GroupNorm32(x_b) * scale + bias                  (bn_stats + mask-matmuls)
  q,k = Wq/Wk (pre-scaled by ch**-0.25, host-transposed) @ h   -> [2 pairs x 128, T] fp16
  vT  = h^T @ WvT_aug  (+bias via K=1 rank-1 matmul; ones-columns appended),
        cast to fp8e4m3 in a DoubleRow-interleaved SBUF layout
  per (tc-chunk, pair, blk of 2 s-tiles):
      ST[s,t] = k^T q  (2 heads packed via PE row-tiling, K=64)    -> PSUM fp32
      pT = exp(ST) -> fp8e4m3: split between ScalarE (table exp) and
           VectorE (Schraudolph bit-trick: int8(ST*8*log2e + bias) bitcast fp8)
      a_aug += vT_aug^T @ pT  via fp8 DoubleRow matmul (K=256 = 2 s-tiles,
           m=65: 64 v-channels + denominator row)
  normalize: reciprocal_approx_fast on the denominator row, K=1 broadcast
      matmul, fused multiply into a_all (B-half partition-shifted via DMA)
  proj partial per t-chunk: WoT_group^T @ a -> host sums the two partials
      per batch + proj_b + x.

Softmax exp uses no max subtraction (logits are O(1): validated range
[-1.35, 1.41]); fp8/Schraudolph errors cancel in the softmax normalization
(validated end-to-end rel err ~2e-4 vs the 2e-2 gate).
"""

import math
import os
import sys

import numpy as np

for _p in ("/opt/trn_rl_repo", "/root/.axon_site/_ro/trn_rl_repo"):
    if _p not in sys.path and os.path.isdir(_p):
        sys.path.insert(0, _p)

B, C, T = 4, 512, 2048
H = 8
G = 32
EPS = 1e-5
CH = C // H          # 64 head dim
NCORES = 8
NKT = C // 128       # 4 contraction tiles
NTT = T // 128       # 16 sequence tiles
NTC = T // 512       # 4 t-chunks
NBLK = NTT // 2      # 8 blocks of 2 s-tiles
QSCALE = 1.0 / math.sqrt(math.sqrt(CH))

# Schraudolph fast-exp constants for fp8e4m3 output:
#   i8 = trunc(ST * 8*log2(e) + (56 + 0.5 - 0.347)); bitcast int8 -> fp8e4m3
C1_EXP8 = 8.0 * math.log2(math.e)
BIAS_EXP8 = 56.0 + 0.5 - 0.347
# fp16 variant: i16 = trunc(ST * 1024*log2(e) + (15360 + 0.5 - 44))
C1_EXP16 = 1024.0 * math.log2(math.e)
BIAS_EXP16 = 15360.0 + 0.5 - 44.0

# which (blk*2 + ab) indices of each (pair, tc) chunk compute exp on the
# VectorE bit-trick instead of ScalarE table-exp (6/16 = 37.5%)
_EXP_MODE = os.environ.get("K_EXP", "mixed")  # mixed | act | dve
if _EXP_MODE == "act":
    DVE_PICK = frozenset()
elif _EXP_MODE == "dve":
    DVE_PICK = frozenset(range(16))
else:
    DVE_PICK = frozenset({1, 4, 7, 9, 12, 15})
PV_FP8 = os.environ.get("K_PV", "fp8") == "fp8"    # fp8 DoubleRow vs fp16 M=65
RECIP_MODE = os.environ.get("K_RECIP", "fast2")    # fast | fast2 | exact
# ("fast" reads PSUM directly, which the custom-uop path miscomputes on HW —
#  fast2 stages the denominator row through SBUF first)

_PROG = None
LAST_RESULT = None


def _build_program():
    import concourse.bass as bass
    import concourse.tile as tile
    from concourse import mybir
    from concourse.bacc import Bacc

    F32 = mybir.dt.float32
    F16 = mybir.dt.float16
    F8 = mybir.dt.float8e4
    I8 = mybir.dt.int8
    I16 = mybir.dt.int16
    AF = mybir.ActivationFunctionType
    OP = mybir.AluOpType

    nc = Bacc(trn_type="TRN2")

    x_d = nc.dram_tensor("x", [4, 128, T], F32, kind="ExternalInput")
    wq_d = nc.dram_tensor("wq", [128, NKT, 256], F16, kind="ExternalInput")
    wk_d = nc.dram_tensor("wk", [128, NKT, 256], F16, kind="ExternalInput")
    wv_d = nc.dram_tensor("wv", [128, NKT, 260], F16, kind="ExternalInput")
    bv_d = nc.dram_tensor("bv", [1, 260], F16, kind="ExternalInput")
    bqk_d = nc.dram_tensor("bqk", [128, 4], F32, kind="ExternalInput")
    wo_d = nc.dram_tensor("wo", [128, 2, 512], F16, kind="ExternalInput")
    gm_d = nc.dram_tensor("gmask", [128, NKT, G], F32, kind="ExternalInput")
    bm_d = nc.dram_tensor("bmask", [G, NKT, 128], F32, kind="ExternalInput")
    gb_d = nc.dram_tensor("gb", [128, NKT, 2], F32, kind="ExternalInput")
    out_d = nc.dram_tensor("out", [4, 128, T], F32, kind="ExternalOutput")

    with tile.TileContext(nc) as tc:
        with (
            tc.tile_pool(name="singles", bufs=1) as singles,
            tc.tile_pool(name="work", bufs=2) as work,
            tc.tile_pool(name="ps", bufs=1, space="PSUM") as ps,
        ):
            # ---- persistent SBUF: weights / constants ----
            wq_sb = singles.tile([128, NKT, 256], F16)
            nc.sync.dma_start(out=wq_sb, in_=wq_d[:, :, :])
            wk_sb = singles.tile([128, NKT, 256], F16)
            nc.sync.dma_start(out=wk_sb, in_=wk_d[:, :, :])
            wv_sb = singles.tile([128, NKT, 260], F16)
            nc.sync.dma_start(out=wv_sb, in_=wv_d[:, :, :])
            bv_sb = singles.tile([1, 260], F16)
            nc.sync.dma_start(out=bv_sb, in_=bv_d[:, :])
            bqk_sb = singles.tile([128, 4], F32)
            nc.sync.dma_start(out=bqk_sb, in_=bqk_d[:, :])
            wo_sb = singles.tile([128, 2, 512], F16)
            nc.sync.dma_start(out=wo_sb, in_=wo_d[:, :, :])
            gm_sb = singles.tile([128, NKT, G], F32)
            nc.sync.dma_start(out=gm_sb, in_=gm_d[:, :, :])
            bm_sb = singles.tile([G, NKT, 128], F32)
            nc.sync.dma_start(out=bm_sb, in_=bm_d[:, :, :])
            gb_sb = singles.tile([128, NKT, 2], F32)
            nc.sync.dma_start(out=gb_sb, in_=gb_d[:, :, :])

            ones1 = singles.tile([1, 128], F16)
            nc.vector.memset(ones1, 1.0)
            onesc = singles.tile([1, 64], F32)
            nc.vector.memset(onesc, 1.0)
            AB = [singles.tile([128, 2], F32, name=f"ab{i}") for i in range(NKT)]
            grp2 = singles.tile([G, 2], F32)
            eps_sb = singles.tile([G, 1], F32)
            nc.vector.memset(eps_sb, EPS)

            # persistent activations
            x_sb = [singles.tile([128, T], F32, name=f"xt{i}") for i in range(NKT)]
            h_sb = [singles.tile([128, T], F16, name=f"ht{i}") for i in range(NKT)]
            q_sb = [singles.tile([128, T], F16, name=f"qp{p}") for p in range(2)]
            k_sb = [singles.tile([128, T], F16, name=f"kp{p}") for p in range(2)]
            # vT in fp8, DoubleRow layout: [s_in_tile, blk, j, (pair,half), 80pad]
            vt8 = singles.tile([128, NBLK, 2, 4, 80], F8)
            nc.vector.memset(vt8, 0.0)
            if not PV_FP8:
                vt16 = singles.tile([128, NTT, 260], F16)
            a_all = [singles.tile([128, T], F16, name=f"aall{p}") for p in range(2)]
            out_sb = [singles.tile([128, T], F32, name=f"os{m}") for m in range(4)]

            for i in range(NKT):
                nc.sync.dma_start(out=x_sb[i], in_=x_d[i])

            # ================= Phase 1: GroupNorm =================
            gs_ps = ps.tile([G, 2], F32, tag="aux", bufs=2, name="gs")
            for i in range(NKT):
                st6 = work.tile([128, 4, 6], F32, tag="st6")
                for sg in range(4):
                    nc.vector.bn_stats(
                        out=st6[:, sg, :], in_=x_sb[i][:, sg * 512 : (sg + 1) * 512]
                    )
                mv = work.tile([128, 2], F32, tag="mv")
                nc.vector.bn_aggr(out=mv, in_=st6)
                s2 = work.tile([128, 2], F32, tag="s2", bufs=4)
                nc.vector.tensor_copy(out=s2[:, 0:1], in_=mv[:, 0:1])
                nc.vector.tensor_mul(out=s2[:, 1:2], in0=mv[:, 0:1], in1=mv[:, 0:1])
                nc.vector.tensor_add(out=s2[:, 1:2], in0=s2[:, 1:2], in1=mv[:, 1:2])
                nc.tensor.matmul(
                    gs_ps, gm_sb[:, i, :], s2, start=(i == 0), stop=(i == NKT - 1)
                )
            gtmp = work.tile([G, 2], F32, tag="gt")
            nc.vector.tensor_scalar_mul(out=gtmp, in0=gs_ps, scalar1=1.0 / 16.0)
            var = work.tile([G, 1], F32, tag="var")
            nc.vector.tensor_mul(out=var, in0=gtmp[:, 0:1], in1=gtmp[:, 0:1])
            nc.vector.tensor_sub(out=var, in0=gtmp[:, 1:2], in1=var)
            # rstd = exp(-0.5 * ln(var + eps))
            nc.scalar.activation(out=var, in_=var, func=AF.Ln, bias=eps_sb)
            nc.scalar.activation(out=grp2[:, 0:1], in_=var, func=AF.Exp, scale=-0.5)
            nc.vector.tensor_copy(out=grp2[:, 1:2], in_=gtmp[:, 0:1])
            for i in range(NKT):
                ch_ps = ps.tile([128, 2], F32, tag="aux", bufs=2, name=f"ch{i}")
                nc.tensor.matmul(ch_ps, bm_sb[:, i, :], grp2, start=True, stop=True)
                nc.vector.tensor_mul(
                    out=AB[i][:, 0:1], in0=ch_ps[:, 0:1], in1=gb_sb[:, i, 0:1]
                )
                t1 = work.tile([128, 1], F32, tag="t1")
                nc.vector.tensor_mul(out=t1, in0=ch_ps[:, 1:2], in1=AB[i][:, 0:1])
                nc.vector.tensor_sub(out=AB[i][:, 1:2], in0=gb_sb[:, i, 1:2], in1=t1)
            for i in range(NKT):
                nc.vector.tensor_scalar(
                    out=h_sb[i],
                    in0=x_sb[i],
                    scalar1=AB[i][:, 0:1],
                    scalar2=AB[i][:, 1:2],
                    op0=OP.mult,
                    op1=OP.add,
                )

            # ================= Phase 2: QKV =================
            for pair in range(2):
                for tcq in range(NTC):
                    q_ps = ps.tile([128, 512], F32, tag="aux", bufs=2, name="qps")
                    for kt in range(NKT):
                        nc.tensor.matmul(
                            q_ps,
                            wq_sb[:, kt, pair * 128 : (pair + 1) * 128],
                            h_sb[kt][:, tcq * 512 : (tcq + 1) * 512],
                            start=(kt == 0),
                            stop=(kt == NKT - 1),
                        )
                    nc.vector.tensor_scalar_add(
                        out=q_sb[pair][:, tcq * 512 : (tcq + 1) * 512],
                        in0=q_ps,
                        scalar1=bqk_sb[:, pair : pair + 1],
                    )
            for pair in range(2):
                for tcq in range(NTC):
                    k_ps = ps.tile([128, 512], F32, tag="aux", bufs=2, name="kps")
                    for kt in range(NKT):
                        nc.tensor.matmul(
                            k_ps,
                            wk_sb[:, kt, pair * 128 : (pair + 1) * 128],
                            h_sb[kt][:, tcq * 512 : (tcq + 1) * 512],
                            start=(kt == 0),
                            stop=(kt == NKT - 1),
                        )
                    nc.vector.tensor_scalar_add(
                        out=k_sb[pair][:, tcq * 512 : (tcq + 1) * 512],
                        in0=k_ps,
                        scalar1=bqk_sb[:, 2 + pair : 3 + pair],
                    )
            # vT (ones columns filled by the K=1 bias matmul), cast to fp8
            for tt in range(NTT):
                vt_ps = ps.tile([128, 260], F32, tag="aux", bufs=2, name="vtps")
                for kt in range(NKT):
                    nc.tensor.matmul(
                        vt_ps,
                        h_sb[kt][:, tt * 128 : (tt + 1) * 128],
                        wv_sb[:, kt, :],
                        start=(kt == 0),
                        stop=False,
                    )
                nc.tensor.matmul(vt_ps, ones1, bv_sb, start=False, stop=True)
                if PV_FP8:
                    nc.vector.tensor_copy(
                        out=vt8[:, tt // 2, tt % 2, :, 0:65],
                        in_=vt_ps.rearrange("p (four s) -> p four s", four=4),
                    )
                else:
                    nc.vector.tensor_copy(out=vt16[:, tt, :], in_=vt_ps)

            # ================= Phase 3: attention + proj =================
            for tcn in range(NTC):
                tcs = slice(tcn * 512, (tcn + 1) * 512)
                for pair in range(2):
                    accA = ps.tile([65, 512], F32, tag="accA", bufs=1)
                    accB = ps.tile([65, 512], F32, tag="accB", bufs=1)
                    for blk in range(NBLK):
                        ST_A = ps.tile([128, 2, 512], F32, tag="st", bufs=2)
                        ST_B = ps.tile([128, 2, 512], F32, tag="st", bufs=2)
                        for j in range(2):
                            sti = blk * 2 + j
                            ss = slice(sti * 128, (sti + 1) * 128)
                            nc.tensor.matmul(
                                ST_A[:, j, :],
                                k_sb[pair][0:64, ss],
                                q_sb[pair][0:64, tcs],
                                start=True,
                                stop=True,
                            )
                            nc.tensor.matmul(
                                ST_B[:, j, :],
                                k_sb[pair][64:128, ss],
                                q_sb[pair][64:128, tcs],
                                start=True,
                                stop=True,
                            )
                        pdt = F8 if PV_FP8 else F16
                        pT_A = work.tile([128, 2, 512], pdt, tag="pt", bufs=6)
                        pT_B = work.tile([128, 2, 512], pdt, tag="pt", bufs=6)
                        for ab, (ST, pT) in enumerate(((ST_A, pT_A), (ST_B, pT_B))):
                            if (blk * 2 + ab) in DVE_PICK:
                                if PV_FP8:
                                    nc.vector.tensor_scalar(
                                        out=pT.bitcast(I8),
                                        in0=ST,
                                        scalar1=C1_EXP8,
                                        scalar2=BIAS_EXP8,
                                        op0=OP.mult,
                                        op1=OP.add,
                                    )
                                else:
                                    nc.vector.tensor_scalar(
                                        out=pT.bitcast(I16),
                                        in0=ST,
                                        scalar1=C1_EXP16,
                                        scalar2=BIAS_EXP16,
                                        op0=OP.mult,
                                        op1=OP.add,
                                    )
                            else:
                                nc.scalar.activation(out=pT, in_=ST, func=AF.Exp)
                        if PV_FP8:
                            # fp8 DoubleRow PV: contract 256 s-positions per mm
                            nc.tensor.matmul(
                                accA,
                                vt8[:, blk, :, 2 * pair, 0:65],
                                pT_A,
                                start=(blk == 0),
                                stop=(blk == NBLK - 1),
                                perf_mode=mybir.MatmulPerfMode.DoubleRow,
                            )
                            nc.tensor.matmul(
                                accB,
                                vt8[:, blk, :, 2 * pair + 1, 0:65],
                                pT_B,
                                start=(blk == 0),
                                stop=(blk == NBLK - 1),
                                perf_mode=mybir.MatmulPerfMode.DoubleRow,
                            )
                        else:
                            for j in range(2):
                                sti = blk * 2 + j
                                nc.tensor.matmul(
                                    accA,
                                    vt16[:, sti, pair * 130 : pair * 130 + 65],
                                    pT_A[:, j, :],
                                    start=(sti == 0),
                                    stop=(sti == NTT - 1),
                                )
                                nc.tensor.matmul(
                                    accB,
                                    vt16[:, sti, pair * 130 + 65 : pair * 130 + 130],
                                    pT_B[:, j, :],
                                    start=(sti == 0),
                                    stop=(sti == NTT - 1),
                                )
                    # normalize by the denominator row (index 64)
                    rdA = work.tile([1, 512], F32, tag="rd", bufs=4)
                    rdB = work.tile([1, 512], F32, tag="rd", bufs=4)
                    if RECIP_MODE == "fast":
                        nc.vector.reciprocal_approx_fast(out=rdA, in_=accA[64:65, :])
                        nc.vector.reciprocal_approx_fast(out=rdB, in_=accB[64:65, :])
                    elif RECIP_MODE == "fast2":
                        dcp = work.tile([1, 2, 512], F32, tag="dcp", bufs=4)
                        nc.vector.tensor_copy(out=dcp[:, 0, :], in_=accA[64:65, :])
                        nc.vector.tensor_copy(out=dcp[:, 1, :], in_=accB[64:65, :])
                        nc.vector.reciprocal_approx_fast(out=rdA, in_=dcp[:, 0, :])
                        nc.vector.reciprocal_approx_fast(out=rdB, in_=dcp[:, 1, :])
                    else:
                        nc.vector.reciprocal(out=rdA, in_=accA[64:65, :])
                        nc.vector.reciprocal(out=rdB, in_=accB[64:65, :])
                    rdbA_ps = ps.tile([64, 512], F32, tag="aux", bufs=2, name="rdba")
                    rdbB_ps = ps.tile([64, 512], F32, tag="aux", bufs=2, name="rdbb")
                    nc.tensor.matmul(rdbA_ps, onesc, rdA, start=True, stop=True)
                    nc.tensor.matmul(rdbB_ps, onesc, rdB, start=True, stop=True)
                    rdbA = work.tile([64, 512], F32, tag="rdb", bufs=4)
                    rdbB = work.tile([64, 512], F32, tag="rdb", bufs=4)
                    nc.scalar.copy(out=rdbA, in_=rdbA_ps)
                    nc.scalar.copy(out=rdbB, in_=rdbB_ps)
                    nc.vector.tensor_mul(
                        out=a_all[pair][0:64, tcs], in0=accA[0:64, :], in1=rdbA
                    )
                    tmpB = work.tile([64, 512], F16, tag="tmpB", bufs=2)
                    nc.vector.tensor_mul(out=tmpB, in0=accB[0:64, :], in1=rdbB)
                    nc.sync.dma_start(out=a_all[pair][64:128, tcs], in_=tmpB)
                # proj partial for this t-chunk
                for m in range(4):
                    op_ps = ps.tile([128, 512], F32, tag="aux", bufs=2, name="opps")
                    for pair in range(2):
                        nc.tensor.matmul(
                            op_ps,
                            wo_sb[:, pair, m * 128 : (m + 1) * 128],
                            a_all[pair][:, tcs],
                            start=(pair == 0),
                            stop=(pair == 1),
                        )
                    nc.vector.tensor_copy(out=out_sb[m][:, tcs], in_=op_ps)
                    if tcn == NTC - 1:
                        nc.sync.dma_start(out=out_d[m], in_=out_sb[m])

    nc.finalize()
    return nc


def _get_program():
    global _PROG
    if _PROG is None:
        _PROG = _build_program()
    return _PROG


def _core_inputs(core, x, norm_scale, norm_bias, qkv_w, qkv_b, proj_w, proj_b):
    b, hg = core // 2, core % 2
    f16 = np.float16
    f32 = np.float32
    hs = slice(hg * 256, hg * 256 + 256)  # head-group channel range

    qw = qkv_w[0:C][hs] * QSCALE          # [256, 512]
    kw = qkv_w[C : 2 * C][hs] * QSCALE
    vw = qkv_w[2 * C : 3 * C][hs]
    qb = qkv_b[0:C][hs] * QSCALE          # [256]
    kb = qkv_b[C : 2 * C][hs] * QSCALE
    vb = qkv_b[2 * C : 3 * C][hs]

    def to_sb_layout(wT):  # [C, 256] -> [128, NKT, 256]
        return np.ascontiguousarray(
            wT.reshape(NKT, 128, 256).transpose(1, 0, 2)
        )

    wq = to_sb_layout(qw.T).astype(f16)
    wk = to_sb_layout(kw.T).astype(f16)

    vwT = vw.T  # [512, 256]
    wv = np.zeros((C, 260), f32)
    bv = np.zeros((1, 260), f32)
    for p in range(2):
        wv[:, p * 130 : p * 130 + 64] = vwT[:, p * 128 : p * 128 + 64]
        wv[:, p * 130 + 65 : p * 130 + 129] = vwT[:, p * 128 + 64 : p * 128 + 128]
        bv[0, p * 130 : p * 130 + 64] = vb[p * 128 : p * 128 + 64]
        bv[0, p * 130 + 64] = 1.0
        bv[0, p * 130 + 65 : p * 130 + 129] = vb[p * 128 + 64 : p * 128 + 128]
        bv[0, p * 130 + 129] = 1.0
    wv = np.ascontiguousarray(wv.reshape(NKT, 128, 260).transpose(1, 0, 2)).astype(f16)
    bv = bv.astype(f16)

    bqk = np.stack(
        [qb[0:128], qb[128:256], kb[0:128], kb[128:256]], axis=1
    ).astype(f32)  # [128, 4]

    woT = proj_w[:, hs].T  # [256, 512]
    wo = np.ascontiguousarray(woT.reshape(2, 128, 512).transpose(1, 0, 2)).astype(f16)

    # GroupNorm masks: channel c (tile i, partition p) belongs to group (i*128+p)//16
    ch_idx = np.arange(C)
    grp_of = ch_idx // 16
    gmask = np.zeros((C, G), f32)
    gmask[ch_idx, grp_of] = 1.0
    gm = np.ascontiguousarray(gmask.reshape(NKT, 128, G).transpose(1, 0, 2))
    bm = np.ascontiguousarray(
        gmask.T.reshape(G, NKT, 128)
    )  # [G, NKT, 128]: bmask[g, i, c]
    gb = np.ascontiguousarray(
        np.stack([norm_scale, norm_bias], axis=1).reshape(NKT, 128, 2).transpose(1, 0, 2)
    ).astype(f32)

    return {
        "x": np.ascontiguousarray(x[b].reshape(NKT, 128, T)).astype(f32),
        "wq": wq,
        "wk": wk,
        "wv": wv,
        "bv": bv,
        "bqk": bqk,
        "wo": wo,
        "gmask": gm,
        "bmask": bm,
        "gb": gb,
    }


def kernel(x, norm_scale, norm_bias, qkv_w, qkv_b, proj_w, proj_b):
    global LAST_RESULT
    x = np.asarray(x, np.float32)
    norm_scale = np.asarray(norm_scale, np.float32)
    norm_bias = np.asarray(norm_bias, np.float32)
    qkv_w = np.asarray(qkv_w, np.float32)
    qkv_b = np.asarray(qkv_b, np.float32)
    proj_w = np.asarray(proj_w, np.float32)
    proj_b = np.asarray(proj_b, np.float32)

    from concourse.bass_utils import run_bass_kernel_spmd

    nc = _get_program()
    in_maps = [
        _core_inputs(c, x, norm_scale, norm_bias, qkv_w, qkv_b, proj_w, proj_b)
        for c in range(NCORES)
    ]
    res = run_bass_kernel_spmd(
        nc,
        in_maps,
        core_ids=list(range(NCORES)),
        trace=bool(int(os.environ.get("KERNEL_TRACE", "0"))),
    )
    LAST_RESULT = res
    out = np.empty((B, C, T), np.float32)
    for b in range(B):
        p0 = res.results[2 * b]["out"].reshape(C, T)
        p1 = res.results[2 * b + 1]["out"].reshape(C, T)
        out[b] = x[b] + proj_b[:, None] + p0 + p1
    return out
